# revision 6
# baseline (speedup 1.0000x reference)
"""MoNet (GMMConv GNN) distributed Trainium2 kernel.

Strategy (8 NeuronCores):
  - Nodes partitioned into 8 contiguous blocks of B=6250 (core m owns dests
    [m*B,(m+1)*B)).  Edges bucketed by (dest block, source half) and padded to
    128-lane tiles, so each core's segment-sum over its dest block is local.
  - Per layer: each core computes its block of xg = h @ Wg in bf16 (rows
    padded to 128 cols = 256B), AllGather -> full bf16 xg table in DRAM, then
    per-edge gather of source rows via SWDGE dma_gather (int16 idx, negative
    trailing idx = skipped pad descriptors), gaussian-weighted segment-sum as
    one-hot bf16 matmuls accumulating in PSUM (dest blocks of 128 nodes),
    fused with the root-weight matmul; epilogue relu+bias+residual in
    transposed layout (h kept in f32, bf16 shadow for matmuls).
  - One-hot selection matrices are built in bulk per dest block with two
    broadcast tensor_tensor ops (is_equal with an iota-row table, then a
    per-tile gauss scale), instead of per-tile per-partition-pointer
    tensor_scalar ops which are slow on DVE.
  - Host does index prep only: degree/dinv, edge sorting/padding, per-core
    edge tables. All O(N*F) and O(E*F) math runs on device.
"""

import sys
from contextlib import ExitStack

import numpy as np

if "/opt/trn_rl_repo" not in sys.path:
    sys.path.insert(0, "/opt/trn_rl_repo")

import ml_dtypes

import concourse.bacc as bacc
import concourse.mybir as mybir
import concourse.tile as tile
from concourse import bass_utils

F32 = mybir.dt.float32
BF16 = mybir.dt.bfloat16
I16 = mybir.dt.int16
AF = mybir.ActivationFunctionType
ALU = mybir.AluOpType

P = 128
EPS = 1e-15


class Cfg:
    def __init__(self, N=50000, E=800000, NFEAT=128, NHID=96, NCLASS=40, NL=2, C=8):
        self.N, self.E, self.NFEAT, self.NHID, self.NCLASS = N, E, NFEAT, NHID, NCLASS
        self.NL, self.C = NL, C
        assert N % C == 0
        self.B = N // C
        self.NBLK = (self.B + P - 1) // P
        self.HALF = N // 2
        self.XGW = 128  # bf16 row padded to 256B


def host_prep_dg(cfg, edge_index, edge_weight):
    """Edges bucketed by (dest block, source half) for int16 dma_gather."""
    N, C, B, NBLK, HALF = cfg.N, cfg.C, cfg.B, cfg.NBLK, cfg.HALF
    row = np.asarray(edge_index[0]).astype(np.int64)
    col = np.asarray(edge_index[1]).astype(np.int64)
    ew = np.asarray(edge_weight).astype(np.float64)
    deg = np.bincount(row, weights=ew, minlength=N).astype(np.float32)
    with np.errstate(divide="ignore"):
        dinv = np.where(deg > 0, 1.0 / np.sqrt(deg.astype(np.float64)), 0.0).astype(np.float32)

    half = (row >= HALF).astype(np.int64)
    core = col // B
    loc = col - core * B
    blk = loc // P
    order = np.lexsort((half, blk, core))
    rs, cs = row[order], col[order]
    hs = half[order]
    core, loc, blk = core[order], loc[order], blk[order]
    dl = (loc - blk * P).astype(np.float32)

    NG = NBLK * 2
    g = blk * 2 + hs  # group within core
    cnt = np.zeros((C, NG), np.int64)
    np.add.at(cnt, (core, g), 1)
    K = ((cnt + P - 1) // P).max(axis=0)  # [NG] tiles per (blk, half)
    toff = np.concatenate([[0], np.cumsum(K)]).astype(np.int64)
    T = int(toff[-1])

    gg = core * NG + g
    gcnt = np.bincount(gg, minlength=C * NG)
    gstart = np.concatenate([[0], np.cumsum(gcnt)])[:-1]
    idx_in_g = np.arange(len(gg)) - gstart[gg]
    lane = (idx_in_g % P).astype(np.int64)
    tloc = idx_in_g // P  # tile within the (blk, half) group
    tcol = (toff[g] + tloc).astype(np.int64)

    uvA = np.zeros((C, P, 2 * T), np.float32)
    dlA = np.full((C, P, T), -1.0, np.float32)  # pad sentinel: never matches iota
    uvA[core, lane, tcol] = dinv[rs]
    uvA[core, lane, T + tcol] = dinv[cs]
    dlA[core, lane, tcol] = dl
    # int16 idx in wrapped-16 layout: flat k = tloc*128 + lane within a call;
    # element k at [k % 16, call_off*8 + k // 16]; pad = row 0 (valid; the
    # dl=-1 sentinel zeroes those lanes in the selection matrix).
    idxA = np.zeros((C, 16, 8 * T), np.int16)
    k = tloc * P + lane
    r16 = (k % 16).astype(np.int64)
    c16 = (toff[g] * 8 + k // 16).astype(np.int64)
    idxA[core, r16, c16] = (rs - hs * HALF).astype(np.int16)
    idxA = np.tile(idxA, (1, 8, 1))  # replicate 16-row block to 128 partitions
    return dict(idxA=idxA, uvA=uvA, dlA=dlA, K=[int(x) for x in K],
                toff=[int(x) for x in toff], T=T)


def build(cfg, prep, scal):
    """Build the SPMD Bass graph. scal: list of per-layer dicts with floats
    wp0, wp1, bp, neg_mu, s2inv."""
    NHID, NCLASS, NFEAT = cfg.NHID, cfg.NCLASS, cfg.NFEAT
    B, NBLK, NL, C, XGW = cfg.B, cfg.NBLK, cfg.NL, cfg.C, cfg.XGW
    T = prep["T"]
    HALF = cfg.HALF
    K2, toff = prep["K"], prep["toff"]
    Kmax = max(max(K2), 1)
    KbMax = max(K2[2 * i] + K2[2 * i + 1] for i in range(NBLK))
    MAXT = 7  # cap descriptors per call under the SWDGE ring size

    nc = bacc.Bacc("TRN2", target_bir_lowering=False, debug=False, num_devices=C)
    hT_in = nc.declare_dram_parameter("hT", [NFEAT, B], BF16, isOutput=False)
    idx_in = nc.declare_dram_parameter("idx16", [P, 8 * T], I16, isOutput=False)
    uv_in = nc.declare_dram_parameter("uv", [P, 2 * T], F32, isOutput=False)
    dl_in = nc.declare_dram_parameter("dl", [P, T], BF16, isOutput=False)
    R_in = nc.declare_dram_parameter("R", [P, P], BF16, isOutput=False)
    id_in = nc.declare_dram_parameter("ident", [P, P], BF16, isOutput=False)
    Wemb_in = nc.declare_dram_parameter("Wemb", [NFEAT, NHID], BF16, isOutput=False)
    Wg_in = nc.declare_dram_parameter("Wg", [NL, NHID, NHID], BF16, isOutput=False)
    Wr_in = nc.declare_dram_parameter("Wr", [NL, NHID, NHID], BF16, isOutput=False)
    Wo_in = nc.declare_dram_parameter("Wo", [NHID, NCLASS], BF16, isOutput=False)
    bemb_in = nc.declare_dram_parameter("bemb", [NHID, 1], F32, isOutput=False)
    bconv_in = nc.declare_dram_parameter("bconv", [NHID, NL], F32, isOutput=False)
    bout_in = nc.declare_dram_parameter("bout", [P, NCLASS], F32, isOutput=False)
    out_ext = nc.declare_dram_parameter("out", [B, NCLASS], F32, isOutput=True)

    from concourse import library_config

    with tile.TileContext(nc) as tc, ExitStack() as ctx:
        nc.gpsimd.load_library(library_config.mlp)
        const = ctx.enter_context(tc.tile_pool(name="const", bufs=1))
        sbp = ctx.enter_context(tc.tile_pool(name="sbp", bufs=3))
        xjp = ctx.enter_context(tc.tile_pool(name="xjp", bufs=6))
        ohp = ctx.enter_context(tc.tile_pool(name="ohp", bufs=2))
        selp = ctx.enter_context(tc.tile_pool(name="selp", bufs=3))
        gp = ctx.enter_context(tc.tile_pool(name="gp", bufs=2))
        gaussp = ctx.enter_context(tc.tile_pool(name="gaussp", bufs=2))
        hp = ctx.enter_context(tc.tile_pool(name="hp", bufs=2))
        hbp = ctx.enter_context(tc.tile_pool(name="hbp", bufs=2))
        pag = ctx.enter_context(tc.tile_pool(name="pag", bufs=3, space="PSUM"))
        pmm = ctx.enter_context(tc.tile_pool(name="pmm", bufs=3, space="PSUM"))
        ptr = ctx.enter_context(tc.tile_pool(name="ptr", bufs=2, space="PSUM"))
        dramp = ctx.enter_context(tc.tile_pool(name="dramp", bufs=1, space="DRAM"))

        def cload(ap, shape, dtype=F32, name=None):
            t = const.tile(shape, dtype, name=name or "c")
            nc.sync.dma_start(out=t[:], in_=ap)
            return t

        hT_s = cload(hT_in[:, :], [NFEAT, B], BF16, name="hT_s")
        idx_s = cload(idx_in[:, :], [P, 8 * T], I16, name="idx_s")
        uv_s = cload(uv_in[:, :], [P, 2 * T], F32, name="uv_s")
        dl_s = cload(dl_in[:, :], [P, T], BF16, name="dl_s")
        u_s = uv_s[:, 0:T]
        v_s = uv_s[:, T:2 * T]
        R_s = cload(R_in[:, :], [P, P], BF16, name="R_s")
        id_s = cload(id_in[:, :], [P, P], BF16, name="id_s")
        Wemb_s = cload(Wemb_in[:, :], [NFEAT, NHID], BF16, name="Wemb_s")
        Wo_s = cload(Wo_in[:, :], [NHID, NCLASS], BF16, name="Wo_s")
        bemb_s = cload(bemb_in[:, :], [NHID, 1], F32, name="bemb_s")
        bconv_s = cload(bconv_in[:, :], [NHID, NL], F32, name="bconv_s")
        bout_s = cload(bout_in[:, :], [P, NCLASS], F32, name="bout_s")
        Wg_s = const.tile([NHID, NL * NHID], BF16, name="Wg_s")
        Wr_s = const.tile([NHID, NL * NHID], BF16, name="Wr_s")
        for i in range(NL):
            nc.sync.dma_start(out=Wg_s[:, i * NHID:(i + 1) * NHID], in_=Wg_in[i])
            nc.sync.dma_start(out=Wr_s[:, i * NHID:(i + 1) * NHID], in_=Wr_in[i])
        bconv_a = const.tile([NHID, NL], F32, name="bconv_a")
        nc.scalar.copy(out=bconv_a[:], in_=bconv_s[:])
        bemb_a = const.tile([NHID, 1], F32, name="bemb_a")
        nc.scalar.copy(out=bemb_a[:], in_=bemb_s[:])
        bout_v = const.tile([P, NCLASS], F32, name="bout_v")
        nc.vector.tensor_copy(out=bout_v[:], in_=bout_s[:])

        def nodeblocks():
            for nt in range(NBLK):
                c0 = nt * P
                yield nt, c0, min(P, B - c0)

        # ---- embedding: h0_T[96, B] = (h @ Wemb + bemb).T ----
        h_cur = hp.tile([NHID, B], F32, tag="h", name="h0")
        hb_cur = hbp.tile([NHID, B], BF16, tag="hb", name="hb0")
        for nt, c0, pn in nodeblocks():
            pe = pmm.tile([P, NHID], F32, tag="mm", name="pe")
            nc.tensor.matmul(pe[:pn, :], lhsT=hT_s[:, c0:c0 + pn], rhs=Wemb_s[:],
                             start=True, stop=True)
            tmp = sbp.tile([P, NHID], BF16, tag="embt", name="embt")
            nc.scalar.copy(out=tmp[:pn, :], in_=pe[:pn, :])
            pt = ptr.tile([NHID, P], BF16, tag="tr", name="pt")
            nc.tensor.transpose(out=pt[:, :pn], in_=tmp[:pn, :NHID], identity=id_s[:pn, :pn])
            nc.scalar.activation(out=h_cur[:, c0:c0 + pn], in_=pt[:, :pn],
                                 func=AF.Identity, bias=bemb_a[:, :1])
            nc.vector.tensor_copy(out=hb_cur[:, c0:c0 + pn], in_=h_cur[:, c0:c0 + pn])

        # ---- layers ----
        for li in range(NL):
            sc = scal[li]
            # xg block (bf16, 256B rows) + all-gather
            xg_src = dramp.tile([B, XGW], BF16, tag="xgs", name=f"xg_src{li}")
            xg_full = dramp.tile([cfg.N, XGW], BF16, tag="xgf", addr_space="Shared",
                                 name=f"xg_full{li}")
            for nt, c0, pn in nodeblocks():
                px = pmm.tile([P, NHID], F32, tag="mm", name="px")
                nc.tensor.matmul(px[:pn, :], lhsT=hb_cur[:, c0:c0 + pn],
                                 rhs=Wg_s[:, li * NHID:(li + 1) * NHID],
                                 start=True, stop=True)
                xs = sbp.tile([P, XGW], BF16, tag="xs", name="xs")
                nc.scalar.copy(out=xs[:pn, :NHID], in_=px[:pn, :])
                nc.sync.dma_start(out=xg_src[c0:c0 + pn, :], in_=xs[:pn, :])
            nc.gpsimd.collective_compute(
                "AllGather", ALU.bypass,
                replica_groups=[list(range(C))],
                ins=[xg_src[:, :]],
                outs=[xg_full[:, :]],
            )

            # gaussian edge coefficients [P, T] (f32 chain, bf16 out)
            t1 = gp.tile([P, T], F32, tag="g1", name="g1")
            t2 = gp.tile([P, T], F32, tag="g2", name="g2")
            nc.vector.tensor_scalar(out=t1[:], in0=u_s[:], scalar1=sc["wp0"],
                                    scalar2=None, op0=ALU.mult)
            nc.vector.tensor_scalar(out=t2[:], in0=v_s[:], scalar1=sc["wp1"],
                                    scalar2=sc["bp"], op0=ALU.mult, op1=ALU.add)
            t3 = gp.tile([P, T], F32, tag="g1", name="g3")
            nc.vector.tensor_tensor(out=t3[:], in0=t1[:], in1=t2[:], op=ALU.add)
            t4 = gp.tile([P, T], F32, tag="g2", name="g4")
            nc.scalar.activation(out=t4[:], in_=t3[:], func=AF.Tanh)
            t4b = gp.tile([P, T], F32, tag="g1", name="g4b")
            nc.vector.tensor_scalar(out=t4b[:], in0=t4[:], scalar1=sc["neg_mu"],
                                    scalar2=None, op0=ALU.add)
            t5 = gp.tile([P, T], F32, tag="g2", name="g5")
            nc.scalar.activation(out=t5[:], in_=t4b[:], func=AF.Square)
            gauss_b = gaussp.tile([P, T], BF16, tag="gauss", name="gauss")
            nc.scalar.activation(out=gauss_b[:], in_=t5[:], func=AF.Exp,
                                 scale=sc["s2inv"])

            # edge aggregation per dest block
            h_new = hp.tile([NHID, B], F32, tag="h", name=f"h{li + 1}")
            hb_new = hbp.tile([NHID, B], BF16, tag="hb", name=f"hb{li + 1}")
            for nt, c0, pn in nodeblocks():
                Kb = K2[2 * nt] + K2[2 * nt + 1]
                off = toff[2 * nt]
                pa = pag.tile([P, NHID], F32, tag="pa", name="pa")
                nc.tensor.matmul(pa[:pn, :], lhsT=hb_cur[:, c0:c0 + pn],
                                 rhs=Wr_s[:, li * NHID:(li + 1) * NHID],
                                 start=True, stop=(Kb == 0))
                if Kb > 0:
                    # bulk one-hot build for all Kb tiles of this dest block
                    oh = ohp.tile([P, KbMax * P], BF16, tag="oh", name="oh")
                    sel = selp.tile([P, KbMax * P], BF16, tag="sel", name="sel")
                    r_b = R_s[:, :].unsqueeze(1).broadcast_to([P, Kb, P])
                    dl_b = dl_s[:, off:off + Kb].unsqueeze(2).broadcast_to([P, Kb, P])
                    g_b = gauss_b[:, off:off + Kb].unsqueeze(2).broadcast_to([P, Kb, P])
                    oh3 = oh[:, :Kb * P].rearrange("p (k e) -> p k e", e=P)
                    sel3 = sel[:, :Kb * P].rearrange("p (k e) -> p k e", e=P)
                    nc.vector.tensor_tensor(out=oh3, in0=r_b, in1=dl_b, op=ALU.is_equal)
                    nc.vector.tensor_tensor(out=sel3, in0=oh3, in1=g_b, op=ALU.mult)
                    kg = 0  # tile index within the block (gather order)
                    for h in (0, 1):
                        Kh = K2[2 * nt + h]
                        if Kh == 0:
                            continue
                        hoff = toff[2 * nt + h]
                        xj = xjp.tile([P, MAXT * XGW], BF16, tag="xj", name="xj")
                        xjs = []
                        for k0 in range(0, Kh, MAXT):
                            kc = min(MAXT, Kh - k0)
                            if k0 > 0:
                                xj = xjp.tile([P, MAXT * XGW], BF16, tag="xj",
                                              name="xj")
                            out_ap = xj[:, 0:kc * XGW].rearrange(
                                "p (k e) -> p k e", e=XGW)
                            nc.gpsimd.dma_gather(
                                out_ap, xg_full[h * HALF:(h + 1) * HALF, :],
                                idx_s[:, (hoff + k0) * 8:(hoff + k0 + kc) * 8],
                                kc * P, kc * P, XGW)
                            xjs.append((xj, kc))
                        for ci, (xj, kc) in enumerate(xjs):
                            for k in range(kc):
                                t = kg + ci * MAXT + k
                                nc.tensor.matmul(
                                    pa[:pn, :],
                                    lhsT=sel[:, t * P:(t + 1) * P][:, :pn],
                                    rhs=xj[:, k * XGW:k * XGW + NHID],
                                    start=False, stop=(t == Kb - 1))
                        kg += Kh
                # epilogue: h_new = h_cur + relu(agg + Wroot h + bconv)
                et = sbp.tile([P, NHID], BF16, tag="et", name="et")
                nc.scalar.copy(out=et[:pn, :], in_=pa[:pn, :])
                pt2 = ptr.tile([NHID, P], BF16, tag="tr", name="pt2")
                nc.tensor.transpose(out=pt2[:, :pn], in_=et[:pn, :NHID],
                                    identity=id_s[:pn, :pn])
                rl = sbp.tile([NHID, P], F32, tag="rl", name="rl")
                nc.scalar.activation(out=rl[:, :pn], in_=pt2[:, :pn], func=AF.Relu,
                                     bias=bconv_a[:, li:li + 1])
                nc.vector.tensor_tensor(out=h_new[:, c0:c0 + pn], in0=rl[:, :pn],
                                        in1=h_cur[:, c0:c0 + pn], op=ALU.add)
                nc.vector.tensor_copy(out=hb_new[:, c0:c0 + pn],
                                      in_=h_new[:, c0:c0 + pn])
            h_cur, hb_cur = h_new, hb_new

        # ---- output head ----
        for nt, c0, pn in nodeblocks():
            po = pmm.tile([P, NHID], F32, tag="mm", name="po")
            nc.tensor.matmul(po[:pn, :NCLASS], lhsT=hb_cur[:, c0:c0 + pn], rhs=Wo_s[:],
                             start=True, stop=True)
            ob = sbp.tile([P, NCLASS], F32, tag="ob", name="ob")
            nc.vector.tensor_tensor(out=ob[:pn, :], in0=po[:pn, :NCLASS],
                                    in1=bout_v[:pn, :], op=ALU.add)
            nc.sync.dma_start(out=out_ext[c0:c0 + pn, :], in_=ob[:pn, :])

    nc.finalize()
    return nc


def make_in_maps(cfg, prep, h, W_emb, b_emb, Wg, Wroot, b_conv, W_out, b_out):
    C, B, NL, NHID = cfg.C, cfg.B, cfg.NL, cfg.NHID
    h = np.asarray(h, np.float32)
    R = np.tile(np.arange(P, dtype=np.float32), (P, 1))
    ident = np.eye(P, dtype=np.float32)
    bf = ml_dtypes.bfloat16
    common = dict(
        R=np.ascontiguousarray(R.astype(bf)),
        ident=np.ascontiguousarray(ident.astype(bf)),
        Wemb=np.ascontiguousarray(np.asarray(W_emb, np.float32).astype(bf)),
        Wg=np.ascontiguousarray(np.asarray(Wg, np.float32).reshape(NL, NHID, NHID).astype(bf)),
        Wr=np.ascontiguousarray(np.asarray(Wroot, np.float32).astype(bf)),
        Wo=np.ascontiguousarray(np.asarray(W_out, np.float32).astype(bf)),
        bemb=np.ascontiguousarray(np.asarray(b_emb, np.float32)[:, None]),
        bconv=np.ascontiguousarray(np.asarray(b_conv, np.float32).T),
        bout=np.ascontiguousarray(np.tile(np.asarray(b_out, np.float32), (P, 1))),
    )
    in_maps = []
    for m in range(C):
        d = dict(common)
        d["hT"] = np.ascontiguousarray(h[m * B:(m + 1) * B, :].T.astype(bf))
        d["idx16"] = np.ascontiguousarray(prep["idxA"][m])
        d["uv"] = np.ascontiguousarray(prep["uvA"][m])
        d["dl"] = np.ascontiguousarray(prep["dlA"][m].astype(bf))
        in_maps.append(d)
    return in_maps


def make_scal(cfg, Wp, bp, mu, sigma):
    Wp = np.asarray(Wp, np.float64)
    bp = np.asarray(bp, np.float64)
    mu = np.asarray(mu, np.float64)
    sigma = np.asarray(sigma, np.float64)
    out = []
    for i in range(cfg.NL):
        out.append(dict(
            wp0=float(Wp[i, 0, 0]),
            wp1=float(Wp[i, 1, 0]),
            bp=float(bp[i, 0]),
            neg_mu=float(-mu[i, 0, 0]),
            s2inv=float(-0.5 / (EPS + sigma[i, 0, 0] ** 2)),
        ))
    return out


def run(cfg, inputs, trace=False):
    prep = host_prep_dg(cfg, inputs["edge_index"], inputs["edge_weight"])
    scal = make_scal(cfg, inputs["Wp"], inputs["bp"], inputs["mu"], inputs["sigma"])
    nc = build(cfg, prep, scal)
    in_maps = make_in_maps(cfg, prep, inputs["h"], inputs["W_emb"], inputs["b_emb"],
                           inputs["Wg"], inputs["Wroot"], inputs["b_conv"],
                           inputs["W_out"], inputs["b_out"])
    res = bass_utils.run_bass_kernel_spmd(nc, in_maps, core_ids=list(range(cfg.C)),
                                          trace=trace)
    out = np.concatenate([res.results[m]["out"] for m in range(cfg.C)], axis=0)
    return out.astype(np.float32), res


def kernel(**inputs):
    cfg = Cfg()
    out, _ = run(cfg, inputs, trace=False)
    return out


# revision 8
# speedup vs baseline: 1.5825x; 1.5825x over previous
"""MoNet (GMMConv GNN) distributed Trainium2 kernel.

Strategy (8 NeuronCores):
  - Nodes partitioned into 8 contiguous blocks of B=6250 (core m owns dests
    [m*B,(m+1)*B)).  Edges bucketed by (dest block, source half) and padded to
    128-lane tiles, so each core's segment-sum over its dest block is local.
  - Per layer: each core computes its block of xg = h @ Wg in bf16 (rows
    padded to 128 cols = 256B), AllGather -> full bf16 xg table in DRAM, then
    per-edge gather of source rows via SWDGE dma_gather (int16 idx, negative
    trailing idx = skipped pad descriptors), gaussian-weighted segment-sum as
    one-hot bf16 matmuls accumulating in PSUM (dest blocks of 128 nodes),
    fused with the root-weight matmul; epilogue relu+bias+residual in
    transposed layout (h kept in f32, bf16 shadow for matmuls).
  - One-hot selection matrices are built in bulk per dest block with two
    broadcast tensor_tensor ops (is_equal with an iota-row table, then a
    per-tile gauss scale), instead of per-tile per-partition-pointer
    tensor_scalar ops which are slow on DVE.
  - Host does index prep only: degree/dinv, edge sorting/padding, per-core
    edge tables. All O(N*F) and O(E*F) math runs on device.
"""

import sys
from contextlib import ExitStack

import numpy as np

if "/opt/trn_rl_repo" not in sys.path:
    sys.path.insert(0, "/opt/trn_rl_repo")

import ml_dtypes

import concourse.bacc as bacc
import concourse.mybir as mybir
import concourse.tile as tile
from concourse import bass_utils

F32 = mybir.dt.float32
BF16 = mybir.dt.bfloat16
I16 = mybir.dt.int16
AF = mybir.ActivationFunctionType
ALU = mybir.AluOpType

P = 128
EPS = 1e-15


class Cfg:
    def __init__(self, N=50000, E=800000, NFEAT=128, NHID=96, NCLASS=40, NL=2, C=8):
        self.N, self.E, self.NFEAT, self.NHID, self.NCLASS = N, E, NFEAT, NHID, NCLASS
        self.NL, self.C = NL, C
        assert N % C == 0
        self.B = N // C
        self.NBLK = (self.B + P - 1) // P
        self.HALF = N // 2
        self.XGW = 128  # bf16 row padded to 256B


def host_prep_dg(cfg, edge_index, edge_weight):
    """Edges bucketed by (dest block, source half) for int16 dma_gather."""
    N, C, B, NBLK, HALF = cfg.N, cfg.C, cfg.B, cfg.NBLK, cfg.HALF
    row = np.asarray(edge_index[0]).astype(np.int64)
    col = np.asarray(edge_index[1]).astype(np.int64)
    ew = np.asarray(edge_weight).astype(np.float64)
    deg = np.bincount(row, weights=ew, minlength=N).astype(np.float32)
    with np.errstate(divide="ignore"):
        dinv = np.where(deg > 0, 1.0 / np.sqrt(deg.astype(np.float64)), 0.0).astype(np.float32)

    half = (row >= HALF).astype(np.int64)
    core = col // B
    loc = col - core * B
    blk = loc // P
    order = np.lexsort((half, blk, core))
    rs, cs = row[order], col[order]
    hs = half[order]
    core, loc, blk = core[order], loc[order], blk[order]
    dl = (loc - blk * P).astype(np.float32)

    NG = NBLK * 2
    g = blk * 2 + hs  # group within core
    cnt = np.zeros((C, NG), np.int64)
    np.add.at(cnt, (core, g), 1)
    K = ((cnt + P - 1) // P).max(axis=0)  # [NG] tiles per (blk, half)
    toff = np.concatenate([[0], np.cumsum(K)]).astype(np.int64)
    T = int(toff[-1])

    gg = core * NG + g
    gcnt = np.bincount(gg, minlength=C * NG)
    gstart = np.concatenate([[0], np.cumsum(gcnt)])[:-1]
    idx_in_g = np.arange(len(gg)) - gstart[gg]
    lane = (idx_in_g % P).astype(np.int64)
    tloc = idx_in_g // P  # tile within the (blk, half) group
    tcol = (toff[g] + tloc).astype(np.int64)

    uvA = np.zeros((C, P, 2 * T), np.float32)
    dlA = np.full((C, P, T), -1.0, np.float32)  # pad sentinel: never matches iota
    uvA[core, lane, tcol] = dinv[rs]
    uvA[core, lane, T + tcol] = dinv[cs]
    dlA[core, lane, tcol] = dl
    # int16 idx in wrapped-16 layout: flat k = tloc*128 + lane within a call;
    # element k at [k % 16, call_off*8 + k // 16]; pad = row 0 (valid; the
    # dl=-1 sentinel zeroes those lanes in the selection matrix).
    idxA = np.zeros((C, 16, 8 * T), np.int16)
    k = tloc * P + lane
    r16 = (k % 16).astype(np.int64)
    c16 = (toff[g] * 8 + k // 16).astype(np.int64)
    idxA[core, r16, c16] = (rs - hs * HALF).astype(np.int16)
    idxA = np.tile(idxA, (1, 8, 1))  # replicate 16-row block to 128 partitions
    return dict(idxA=idxA, uvA=uvA, dlA=dlA, K=[int(x) for x in K],
                toff=[int(x) for x in toff], T=T)


def build(cfg, prep, scal):
    """Build the SPMD Bass graph. scal: list of per-layer dicts with floats
    wp0, wp1, bp, neg_mu, s2inv."""
    NHID, NCLASS, NFEAT = cfg.NHID, cfg.NCLASS, cfg.NFEAT
    B, NBLK, NL, C, XGW = cfg.B, cfg.NBLK, cfg.NL, cfg.C, cfg.XGW
    T = prep["T"]
    HALF = cfg.HALF
    K2, toff = prep["K"], prep["toff"]
    Kmax = max(max(K2), 1)
    KbMax = max(K2[2 * i] + K2[2 * i + 1] for i in range(NBLK))
    MAXT = 7  # cap descriptors per call under the SWDGE ring size

    nc = bacc.Bacc("TRN2", target_bir_lowering=False, debug=False, num_devices=C,
                   num_swdge_queues=2)
    hT_in = nc.declare_dram_parameter("hT", [NFEAT, B], BF16, isOutput=False)
    idx_in = nc.declare_dram_parameter("idx16", [P, 8 * T], I16, isOutput=False)
    uv_in = nc.declare_dram_parameter("uv", [P, 2 * T], F32, isOutput=False)
    dl_in = nc.declare_dram_parameter("dl", [P, T], BF16, isOutput=False)
    R_in = nc.declare_dram_parameter("R", [P, P], BF16, isOutput=False)
    id_in = nc.declare_dram_parameter("ident", [P, P], BF16, isOutput=False)
    Wemb_in = nc.declare_dram_parameter("Wemb", [NFEAT, NHID], BF16, isOutput=False)
    Wg_in = nc.declare_dram_parameter("Wg", [NL, NHID, NHID], BF16, isOutput=False)
    Wr_in = nc.declare_dram_parameter("Wr", [NL, NHID, NHID], BF16, isOutput=False)
    Wo_in = nc.declare_dram_parameter("Wo", [NHID, NCLASS], BF16, isOutput=False)
    bemb_in = nc.declare_dram_parameter("bemb", [NHID, 1], F32, isOutput=False)
    bconv_in = nc.declare_dram_parameter("bconv", [NHID, NL], F32, isOutput=False)
    bout_in = nc.declare_dram_parameter("bout", [P, NCLASS], F32, isOutput=False)
    out_ext = nc.declare_dram_parameter("out", [B, NCLASS], F32, isOutput=True)

    from concourse import library_config

    with tile.TileContext(nc) as tc, ExitStack() as ctx:
        nc.gpsimd.load_library(library_config.mlp)
        const = ctx.enter_context(tc.tile_pool(name="const", bufs=1))
        sbp = ctx.enter_context(tc.tile_pool(name="sbp", bufs=3))
        xjp = ctx.enter_context(tc.tile_pool(name="xjp", bufs=6))
        ohp = ctx.enter_context(tc.tile_pool(name="ohp", bufs=2))
        selp = ctx.enter_context(tc.tile_pool(name="selp", bufs=3))
        gp = ctx.enter_context(tc.tile_pool(name="gp", bufs=2))
        gaussp = ctx.enter_context(tc.tile_pool(name="gaussp", bufs=2))
        hp = ctx.enter_context(tc.tile_pool(name="hp", bufs=2))
        hbp = ctx.enter_context(tc.tile_pool(name="hbp", bufs=2))
        pag = ctx.enter_context(tc.tile_pool(name="pag", bufs=3, space="PSUM"))
        pmm = ctx.enter_context(tc.tile_pool(name="pmm", bufs=3, space="PSUM"))
        ptr = ctx.enter_context(tc.tile_pool(name="ptr", bufs=2, space="PSUM"))
        dramp = ctx.enter_context(tc.tile_pool(name="dramp", bufs=1, space="DRAM"))

        def cload(ap, shape, dtype=F32, name=None):
            t = const.tile(shape, dtype, name=name or "c")
            nc.sync.dma_start(out=t[:], in_=ap)
            return t

        hT_s = cload(hT_in[:, :], [NFEAT, B], BF16, name="hT_s")
        idx_s = cload(idx_in[:, :], [P, 8 * T], I16, name="idx_s")
        uv_s = cload(uv_in[:, :], [P, 2 * T], F32, name="uv_s")
        dl_s = cload(dl_in[:, :], [P, T], BF16, name="dl_s")
        u_s = uv_s[:, 0:T]
        v_s = uv_s[:, T:2 * T]
        R_s = cload(R_in[:, :], [P, P], BF16, name="R_s")
        id_s = cload(id_in[:, :], [P, P], BF16, name="id_s")
        Wemb_s = cload(Wemb_in[:, :], [NFEAT, NHID], BF16, name="Wemb_s")
        Wo_s = cload(Wo_in[:, :], [NHID, NCLASS], BF16, name="Wo_s")
        bemb_s = cload(bemb_in[:, :], [NHID, 1], F32, name="bemb_s")
        bconv_s = cload(bconv_in[:, :], [NHID, NL], F32, name="bconv_s")
        bout_s = cload(bout_in[:, :], [P, NCLASS], F32, name="bout_s")
        Wg_s = const.tile([NHID, NL * NHID], BF16, name="Wg_s")
        Wr_s = const.tile([NHID, NL * NHID], BF16, name="Wr_s")
        for i in range(NL):
            nc.sync.dma_start(out=Wg_s[:, i * NHID:(i + 1) * NHID], in_=Wg_in[i])
            nc.sync.dma_start(out=Wr_s[:, i * NHID:(i + 1) * NHID], in_=Wr_in[i])
        bconv_a = const.tile([NHID, NL], F32, name="bconv_a")
        nc.scalar.copy(out=bconv_a[:], in_=bconv_s[:])
        bemb_a = const.tile([NHID, 1], F32, name="bemb_a")
        nc.scalar.copy(out=bemb_a[:], in_=bemb_s[:])
        bout_v = const.tile([P, NCLASS], F32, name="bout_v")
        nc.vector.tensor_copy(out=bout_v[:], in_=bout_s[:])

        def nodeblocks():
            for nt in range(NBLK):
                c0 = nt * P
                yield nt, c0, min(P, B - c0)

        # ---- embedding: h0_T[96, B] = (h @ Wemb + bemb).T ----
        h_cur = hp.tile([NHID, B], F32, tag="h", name="h0")
        hb_cur = hbp.tile([NHID, B], BF16, tag="hb", name="hb0")
        for nt, c0, pn in nodeblocks():
            pe = pmm.tile([P, NHID], F32, tag="mm", name="pe")
            nc.tensor.matmul(pe[:pn, :], lhsT=hT_s[:, c0:c0 + pn], rhs=Wemb_s[:],
                             start=True, stop=True)
            tmp = sbp.tile([P, NHID], BF16, tag="embt", name="embt")
            nc.scalar.copy(out=tmp[:pn, :], in_=pe[:pn, :])
            pt = ptr.tile([NHID, P], BF16, tag="tr", name="pt")
            nc.tensor.transpose(out=pt[:, :pn], in_=tmp[:pn, :NHID], identity=id_s[:pn, :pn])
            nc.scalar.activation(out=h_cur[:, c0:c0 + pn], in_=pt[:, :pn],
                                 func=AF.Identity, bias=bemb_a[:, :1])
            nc.vector.tensor_copy(out=hb_cur[:, c0:c0 + pn], in_=h_cur[:, c0:c0 + pn])

        # ---- layers ----
        for li in range(NL):
            sc = scal[li]
            # xg block (bf16, 256B rows) + all-gather
            xg_src = dramp.tile([B, XGW], BF16, tag="xgs", name=f"xg_src{li}")
            xg_full = dramp.tile([cfg.N, XGW], BF16, tag="xgf", addr_space="Shared",
                                 name=f"xg_full{li}")
            for nt, c0, pn in nodeblocks():
                px = pmm.tile([P, NHID], F32, tag="mm", name="px")
                nc.tensor.matmul(px[:pn, :], lhsT=hb_cur[:, c0:c0 + pn],
                                 rhs=Wg_s[:, li * NHID:(li + 1) * NHID],
                                 start=True, stop=True)
                xs = sbp.tile([P, XGW], BF16, tag="xs", name="xs")
                nc.scalar.copy(out=xs[:pn, :NHID], in_=px[:pn, :])
                nc.sync.dma_start(out=xg_src[c0:c0 + pn, :], in_=xs[:pn, :])
            nc.gpsimd.collective_compute(
                "AllGather", ALU.bypass,
                replica_groups=[list(range(C))],
                ins=[xg_src[:, :]],
                outs=[xg_full[:, :]],
            )

            # gaussian edge coefficients [P, T] (f32 chain, bf16 out)
            t1 = gp.tile([P, T], F32, tag="g1", name="g1")
            t2 = gp.tile([P, T], F32, tag="g2", name="g2")
            nc.vector.tensor_scalar(out=t1[:], in0=u_s[:], scalar1=sc["wp0"],
                                    scalar2=None, op0=ALU.mult)
            nc.vector.tensor_scalar(out=t2[:], in0=v_s[:], scalar1=sc["wp1"],
                                    scalar2=sc["bp"], op0=ALU.mult, op1=ALU.add)
            t3 = gp.tile([P, T], F32, tag="g1", name="g3")
            nc.vector.tensor_tensor(out=t3[:], in0=t1[:], in1=t2[:], op=ALU.add)
            t4 = gp.tile([P, T], F32, tag="g2", name="g4")
            nc.scalar.activation(out=t4[:], in_=t3[:], func=AF.Tanh)
            t4b = gp.tile([P, T], F32, tag="g1", name="g4b")
            nc.vector.tensor_scalar(out=t4b[:], in0=t4[:], scalar1=sc["neg_mu"],
                                    scalar2=None, op0=ALU.add)
            t5 = gp.tile([P, T], F32, tag="g2", name="g5")
            nc.scalar.activation(out=t5[:], in_=t4b[:], func=AF.Square)
            gauss_b = gaussp.tile([P, T], BF16, tag="gauss", name="gauss")
            nc.scalar.activation(out=gauss_b[:], in_=t5[:], func=AF.Exp,
                                 scale=sc["s2inv"])

            # edge aggregation per dest block
            h_new = hp.tile([NHID, B], F32, tag="h", name=f"h{li + 1}")
            hb_new = hbp.tile([NHID, B], BF16, tag="hb", name=f"hb{li + 1}")
            for nt, c0, pn in nodeblocks():
                Kb = K2[2 * nt] + K2[2 * nt + 1]
                off = toff[2 * nt]
                pa = pag.tile([P, NHID], F32, tag="pa", name="pa")
                nc.tensor.matmul(pa[:pn, :], lhsT=hb_cur[:, c0:c0 + pn],
                                 rhs=Wr_s[:, li * NHID:(li + 1) * NHID],
                                 start=True, stop=(Kb == 0))
                if Kb > 0:
                    # bulk one-hot build for all Kb tiles of this dest block
                    oh = ohp.tile([P, KbMax * P], BF16, tag="oh", name="oh")
                    sel = selp.tile([P, KbMax * P], BF16, tag="sel", name="sel")
                    r_b = R_s[:, :].unsqueeze(1).broadcast_to([P, Kb, P])
                    dl_b = dl_s[:, off:off + Kb].unsqueeze(2).broadcast_to([P, Kb, P])
                    g_b = gauss_b[:, off:off + Kb].unsqueeze(2).broadcast_to([P, Kb, P])
                    oh3 = oh[:, :Kb * P].rearrange("p (k e) -> p k e", e=P)
                    sel3 = sel[:, :Kb * P].rearrange("p (k e) -> p k e", e=P)
                    nc.vector.tensor_tensor(out=oh3, in0=r_b, in1=dl_b, op=ALU.is_equal)
                    nc.vector.tensor_tensor(out=sel3, in0=oh3, in1=g_b, op=ALU.mult)
                    kg = 0  # tile index within the block (gather order)
                    for h in (0, 1):
                        Kh = K2[2 * nt + h]
                        if Kh == 0:
                            continue
                        hoff = toff[2 * nt + h]
                        xj = xjp.tile([P, MAXT * XGW], BF16, tag="xj", name="xj")
                        xjs = []
                        for k0 in range(0, Kh, MAXT):
                            kc = min(MAXT, Kh - k0)
                            if k0 > 0:
                                xj = xjp.tile([P, MAXT * XGW], BF16, tag="xj",
                                              name="xj")
                            out_ap = xj[:, 0:kc * XGW].rearrange(
                                "p (k e) -> p k e", e=XGW)
                            nc.gpsimd.dma_gather(
                                out_ap, xg_full[h * HALF:(h + 1) * HALF, :],
                                idx_s[:, (hoff + k0) * 8:(hoff + k0 + kc) * 8],
                                kc * P, kc * P, XGW,
                                queue_num=(2 * nt + h) % 2)
                            xjs.append((xj, kc))
                        for ci, (xj, kc) in enumerate(xjs):
                            for k in range(kc):
                                t = kg + ci * MAXT + k
                                nc.tensor.matmul(
                                    pa[:pn, :],
                                    lhsT=sel[:, t * P:(t + 1) * P][:, :pn],
                                    rhs=xj[:, k * XGW:k * XGW + NHID],
                                    start=False, stop=(t == Kb - 1))
                        kg += Kh
                # epilogue: h_new = h_cur + relu(agg + Wroot h + bconv)
                et = sbp.tile([P, NHID], BF16, tag="et", name="et")
                nc.scalar.copy(out=et[:pn, :], in_=pa[:pn, :])
                pt2 = ptr.tile([NHID, P], BF16, tag="tr", name="pt2")
                nc.tensor.transpose(out=pt2[:, :pn], in_=et[:pn, :NHID],
                                    identity=id_s[:pn, :pn])
                rl = sbp.tile([NHID, P], F32, tag="rl", name="rl")
                nc.scalar.activation(out=rl[:, :pn], in_=pt2[:, :pn], func=AF.Relu,
                                     bias=bconv_a[:, li:li + 1])
                nc.vector.tensor_tensor(out=h_new[:, c0:c0 + pn], in0=rl[:, :pn],
                                        in1=h_cur[:, c0:c0 + pn], op=ALU.add)
                nc.vector.tensor_copy(out=hb_new[:, c0:c0 + pn],
                                      in_=h_new[:, c0:c0 + pn])
            h_cur, hb_cur = h_new, hb_new

        # ---- output head ----
        for nt, c0, pn in nodeblocks():
            po = pmm.tile([P, NHID], F32, tag="mm", name="po")
            nc.tensor.matmul(po[:pn, :NCLASS], lhsT=hb_cur[:, c0:c0 + pn], rhs=Wo_s[:],
                             start=True, stop=True)
            ob = sbp.tile([P, NCLASS], F32, tag="ob", name="ob")
            nc.vector.tensor_tensor(out=ob[:pn, :], in0=po[:pn, :NCLASS],
                                    in1=bout_v[:pn, :], op=ALU.add)
            nc.sync.dma_start(out=out_ext[c0:c0 + pn, :], in_=ob[:pn, :])

    nc.finalize()
    return nc


def make_in_maps(cfg, prep, h, W_emb, b_emb, Wg, Wroot, b_conv, W_out, b_out):
    C, B, NL, NHID = cfg.C, cfg.B, cfg.NL, cfg.NHID
    h = np.asarray(h, np.float32)
    R = np.tile(np.arange(P, dtype=np.float32), (P, 1))
    ident = np.eye(P, dtype=np.float32)
    bf = ml_dtypes.bfloat16
    common = dict(
        R=np.ascontiguousarray(R.astype(bf)),
        ident=np.ascontiguousarray(ident.astype(bf)),
        Wemb=np.ascontiguousarray(np.asarray(W_emb, np.float32).astype(bf)),
        Wg=np.ascontiguousarray(np.asarray(Wg, np.float32).reshape(NL, NHID, NHID).astype(bf)),
        Wr=np.ascontiguousarray(np.asarray(Wroot, np.float32).astype(bf)),
        Wo=np.ascontiguousarray(np.asarray(W_out, np.float32).astype(bf)),
        bemb=np.ascontiguousarray(np.asarray(b_emb, np.float32)[:, None]),
        bconv=np.ascontiguousarray(np.asarray(b_conv, np.float32).T),
        bout=np.ascontiguousarray(np.tile(np.asarray(b_out, np.float32), (P, 1))),
    )
    in_maps = []
    for m in range(C):
        d = dict(common)
        d["hT"] = np.ascontiguousarray(h[m * B:(m + 1) * B, :].T.astype(bf))
        d["idx16"] = np.ascontiguousarray(prep["idxA"][m])
        d["uv"] = np.ascontiguousarray(prep["uvA"][m])
        d["dl"] = np.ascontiguousarray(prep["dlA"][m].astype(bf))
        in_maps.append(d)
    return in_maps


def make_scal(cfg, Wp, bp, mu, sigma):
    Wp = np.asarray(Wp, np.float64)
    bp = np.asarray(bp, np.float64)
    mu = np.asarray(mu, np.float64)
    sigma = np.asarray(sigma, np.float64)
    out = []
    for i in range(cfg.NL):
        out.append(dict(
            wp0=float(Wp[i, 0, 0]),
            wp1=float(Wp[i, 1, 0]),
            bp=float(bp[i, 0]),
            neg_mu=float(-mu[i, 0, 0]),
            s2inv=float(-0.5 / (EPS + sigma[i, 0, 0] ** 2)),
        ))
    return out


def run(cfg, inputs, trace=False):
    prep = host_prep_dg(cfg, inputs["edge_index"], inputs["edge_weight"])
    scal = make_scal(cfg, inputs["Wp"], inputs["bp"], inputs["mu"], inputs["sigma"])
    nc = build(cfg, prep, scal)
    in_maps = make_in_maps(cfg, prep, inputs["h"], inputs["W_emb"], inputs["b_emb"],
                           inputs["Wg"], inputs["Wroot"], inputs["b_conv"],
                           inputs["W_out"], inputs["b_out"])
    res = bass_utils.run_bass_kernel_spmd(nc, in_maps, core_ids=list(range(cfg.C)),
                                          trace=trace)
    out = np.concatenate([res.results[m]["out"] for m in range(cfg.C)], axis=0)
    return out.astype(np.float32), res


def kernel(**inputs):
    cfg = Cfg()
    out, _ = run(cfg, inputs, trace=False)
    return out


# revision 10
# speedup vs baseline: 1.7423x; 1.1010x over previous
"""MoNet (GMMConv GNN) distributed Trainium2 kernel.

Strategy (8 NeuronCores):
  - Nodes partitioned into 8 contiguous blocks of B=6250 (core m owns dests
    [m*B,(m+1)*B)).  Edges bucketed by (dest block, source half) and padded to
    128-lane tiles, so each core's segment-sum over its dest block is local.
  - Per layer: each core computes its block of xg = h @ Wg in bf16 (rows
    padded to 128 cols = 256B), AllGather -> full bf16 xg table in DRAM, then
    per-edge gather of source rows via SWDGE dma_gather (int16 idx, negative
    trailing idx = skipped pad descriptors), gaussian-weighted segment-sum as
    one-hot bf16 matmuls accumulating in PSUM (dest blocks of 128 nodes),
    fused with the root-weight matmul; epilogue relu+bias+residual in
    transposed layout (h kept in f32, bf16 shadow for matmuls).
  - One-hot selection matrices are built in bulk per dest block with two
    broadcast tensor_tensor ops (is_equal with an iota-row table, then a
    per-tile gauss scale), instead of per-tile per-partition-pointer
    tensor_scalar ops which are slow on DVE.
  - Host does index prep only: degree/dinv, edge sorting/padding, per-core
    edge tables. All O(N*F) and O(E*F) math runs on device.
"""

import sys
from contextlib import ExitStack

import numpy as np

if "/opt/trn_rl_repo" not in sys.path:
    sys.path.insert(0, "/opt/trn_rl_repo")

import ml_dtypes

import concourse.bacc as bacc
import concourse.mybir as mybir
import concourse.tile as tile
from concourse import bass_utils

F32 = mybir.dt.float32
BF16 = mybir.dt.bfloat16
I16 = mybir.dt.int16
AF = mybir.ActivationFunctionType
ALU = mybir.AluOpType

P = 128
EPS = 1e-15


class Cfg:
    def __init__(self, N=50000, E=800000, NFEAT=128, NHID=96, NCLASS=40, NL=2, C=8):
        self.N, self.E, self.NFEAT, self.NHID, self.NCLASS = N, E, NFEAT, NHID, NCLASS
        self.NL, self.C = NL, C
        assert N % C == 0
        self.B = N // C
        self.NBLK = (self.B + P - 1) // P
        self.HALF = N // 2
        self.XGW = 128  # bf16 row padded to 256B


def host_prep_dg(cfg, edge_index, edge_weight):
    """Edges bucketed by (dest block, source half) for int16 dma_gather."""
    N, C, B, NBLK, HALF = cfg.N, cfg.C, cfg.B, cfg.NBLK, cfg.HALF
    row = np.asarray(edge_index[0]).astype(np.int64)
    col = np.asarray(edge_index[1]).astype(np.int64)
    ew = np.asarray(edge_weight).astype(np.float64)
    deg = np.bincount(row, weights=ew, minlength=N).astype(np.float32)
    with np.errstate(divide="ignore"):
        dinv = np.where(deg > 0, 1.0 / np.sqrt(deg.astype(np.float64)), 0.0).astype(np.float32)

    half = (row >= HALF).astype(np.int64)
    core = col // B
    loc = col - core * B
    blk = loc // P
    order = np.lexsort((half, blk, core))
    rs, cs = row[order], col[order]
    hs = half[order]
    core, loc, blk = core[order], loc[order], blk[order]
    dl = (loc - blk * P).astype(np.float32)

    NG = NBLK * 2
    g = blk * 2 + hs  # group within core
    cnt = np.zeros((C, NG), np.int64)
    np.add.at(cnt, (core, g), 1)
    K = ((cnt + P - 1) // P).max(axis=0)  # [NG] tiles per (blk, half)
    toff = np.concatenate([[0], np.cumsum(K)]).astype(np.int64)
    T = int(toff[-1])

    gg = core * NG + g
    gcnt = np.bincount(gg, minlength=C * NG)
    gstart = np.concatenate([[0], np.cumsum(gcnt)])[:-1]
    idx_in_g = np.arange(len(gg)) - gstart[gg]
    lane = (idx_in_g % P).astype(np.int64)
    tloc = idx_in_g // P  # tile within the (blk, half) group
    tcol = (toff[g] + tloc).astype(np.int64)

    uvA = np.zeros((C, P, 2 * T), np.float32)
    dlA = np.full((C, P, T), -1.0, np.float32)  # pad sentinel: never matches iota
    uvA[core, lane, tcol] = dinv[rs]
    uvA[core, lane, T + tcol] = dinv[cs]
    dlA[core, lane, tcol] = dl
    # int16 idx in wrapped-16 layout: flat k = tloc*128 + lane within a call;
    # element k at [k % 16, call_off*8 + k // 16]; pad = row 0 (valid; the
    # dl=-1 sentinel zeroes those lanes in the selection matrix).
    idxA = np.zeros((C, 16, 8 * T), np.int16)
    k = tloc * P + lane
    r16 = (k % 16).astype(np.int64)
    c16 = (toff[g] * 8 + k // 16).astype(np.int64)
    idxA[core, r16, c16] = (rs - hs * HALF).astype(np.int16)
    idxA = np.tile(idxA, (1, 8, 1))  # replicate 16-row block to 128 partitions
    return dict(idxA=idxA, uvA=uvA, dlA=dlA, K=[int(x) for x in K],
                toff=[int(x) for x in toff], T=T)


def build(cfg, prep, scal):
    """Build the SPMD Bass graph. scal: list of per-layer dicts with floats
    wp0, wp1, bp, neg_mu, s2inv."""
    NHID, NCLASS, NFEAT = cfg.NHID, cfg.NCLASS, cfg.NFEAT
    B, NBLK, NL, C, XGW = cfg.B, cfg.NBLK, cfg.NL, cfg.C, cfg.XGW
    T = prep["T"]
    HALF = cfg.HALF
    K2, toff = prep["K"], prep["toff"]
    Kmax = max(max(K2), 1)
    KbMax = max(K2[2 * i] + K2[2 * i + 1] for i in range(NBLK))
    MAXT = 7  # cap descriptors per call under the SWDGE ring size

    nc = bacc.Bacc("TRN2", target_bir_lowering=False, debug=False, num_devices=C,
                   num_swdge_queues=4)
    hT_in = nc.declare_dram_parameter("hT", [NFEAT, B], BF16, isOutput=False)
    idx_in = nc.declare_dram_parameter("idx16", [P, 8 * T], I16, isOutput=False)
    uv_in = nc.declare_dram_parameter("uv", [P, 2 * T], F32, isOutput=False)
    dl_in = nc.declare_dram_parameter("dl", [P, T], BF16, isOutput=False)
    R_in = nc.declare_dram_parameter("R", [P, P], BF16, isOutput=False)
    id_in = nc.declare_dram_parameter("ident", [P, P], BF16, isOutput=False)
    Wemb_in = nc.declare_dram_parameter("Wemb", [NFEAT, NHID], BF16, isOutput=False)
    Wg_in = nc.declare_dram_parameter("Wg", [NL, NHID, NHID], BF16, isOutput=False)
    Wr_in = nc.declare_dram_parameter("Wr", [NL, NHID, NHID], BF16, isOutput=False)
    Wo_in = nc.declare_dram_parameter("Wo", [NHID, NCLASS], BF16, isOutput=False)
    bemb_in = nc.declare_dram_parameter("bemb", [NHID, 1], F32, isOutput=False)
    bconv_in = nc.declare_dram_parameter("bconv", [NHID, NL], F32, isOutput=False)
    bout_in = nc.declare_dram_parameter("bout", [P, NCLASS], F32, isOutput=False)
    out_ext = nc.declare_dram_parameter("out", [B, NCLASS], F32, isOutput=True)

    from concourse import library_config

    with tile.TileContext(nc) as tc, ExitStack() as ctx:
        nc.gpsimd.load_library(library_config.mlp)
        const = ctx.enter_context(tc.tile_pool(name="const", bufs=1))
        sbp = ctx.enter_context(tc.tile_pool(name="sbp", bufs=3))
        xjp = ctx.enter_context(tc.tile_pool(name="xjp", bufs=6))
        ohp = ctx.enter_context(tc.tile_pool(name="ohp", bufs=2))
        selp = ctx.enter_context(tc.tile_pool(name="selp", bufs=3))
        gp = ctx.enter_context(tc.tile_pool(name="gp", bufs=2))
        gaussp = ctx.enter_context(tc.tile_pool(name="gaussp", bufs=2))
        hp = ctx.enter_context(tc.tile_pool(name="hp", bufs=2))
        hbp = ctx.enter_context(tc.tile_pool(name="hbp", bufs=2))
        pag = ctx.enter_context(tc.tile_pool(name="pag", bufs=3, space="PSUM"))
        pmm = ctx.enter_context(tc.tile_pool(name="pmm", bufs=3, space="PSUM"))
        ptr = ctx.enter_context(tc.tile_pool(name="ptr", bufs=2, space="PSUM"))
        dramp = ctx.enter_context(tc.tile_pool(name="dramp", bufs=1, space="DRAM"))

        def cload(ap, shape, dtype=F32, name=None):
            t = const.tile(shape, dtype, name=name or "c")
            nc.sync.dma_start(out=t[:], in_=ap)
            return t

        hT_s = cload(hT_in[:, :], [NFEAT, B], BF16, name="hT_s")
        idx_s = cload(idx_in[:, :], [P, 8 * T], I16, name="idx_s")
        uv_s = cload(uv_in[:, :], [P, 2 * T], F32, name="uv_s")
        dl_s = cload(dl_in[:, :], [P, T], BF16, name="dl_s")
        u_s = uv_s[:, 0:T]
        v_s = uv_s[:, T:2 * T]
        R_s = cload(R_in[:, :], [P, P], BF16, name="R_s")
        id_s = cload(id_in[:, :], [P, P], BF16, name="id_s")
        Wemb_s = cload(Wemb_in[:, :], [NFEAT, NHID], BF16, name="Wemb_s")
        Wo_s = cload(Wo_in[:, :], [NHID, NCLASS], BF16, name="Wo_s")
        bemb_s = cload(bemb_in[:, :], [NHID, 1], F32, name="bemb_s")
        bconv_s = cload(bconv_in[:, :], [NHID, NL], F32, name="bconv_s")
        bout_s = cload(bout_in[:, :], [P, NCLASS], F32, name="bout_s")
        Wg_s = const.tile([NHID, NL * NHID], BF16, name="Wg_s")
        Wr_s = const.tile([NHID, NL * NHID], BF16, name="Wr_s")
        for i in range(NL):
            nc.sync.dma_start(out=Wg_s[:, i * NHID:(i + 1) * NHID], in_=Wg_in[i])
            nc.sync.dma_start(out=Wr_s[:, i * NHID:(i + 1) * NHID], in_=Wr_in[i])
        bconv_a = const.tile([NHID, NL], F32, name="bconv_a")
        nc.scalar.copy(out=bconv_a[:], in_=bconv_s[:])
        bemb_a = const.tile([NHID, 1], F32, name="bemb_a")
        nc.scalar.copy(out=bemb_a[:], in_=bemb_s[:])
        bout_v = const.tile([P, NCLASS], F32, name="bout_v")
        nc.vector.tensor_copy(out=bout_v[:], in_=bout_s[:])

        def nodeblocks():
            for nt in range(NBLK):
                c0 = nt * P
                yield nt, c0, min(P, B - c0)

        # ---- embedding: h0_T[96, B] = (h @ Wemb + bemb).T ----
        h_cur = hp.tile([NHID, B], F32, tag="h", name="h0")
        hb_cur = hbp.tile([NHID, B], BF16, tag="hb", name="hb0")
        for nt, c0, pn in nodeblocks():
            pe = pmm.tile([P, NHID], F32, tag="mm", name="pe")
            nc.tensor.matmul(pe[:pn, :], lhsT=hT_s[:, c0:c0 + pn], rhs=Wemb_s[:],
                             start=True, stop=True)
            tmp = sbp.tile([P, NHID], BF16, tag="embt", name="embt")
            nc.scalar.copy(out=tmp[:pn, :], in_=pe[:pn, :])
            pt = ptr.tile([NHID, P], BF16, tag="tr", name="pt")
            nc.tensor.transpose(out=pt[:, :pn], in_=tmp[:pn, :NHID], identity=id_s[:pn, :pn])
            nc.scalar.activation(out=h_cur[:, c0:c0 + pn], in_=pt[:, :pn],
                                 func=AF.Identity, bias=bemb_a[:, :1])
            nc.vector.tensor_copy(out=hb_cur[:, c0:c0 + pn], in_=h_cur[:, c0:c0 + pn])

        # ---- layers ----
        for li in range(NL):
            sc = scal[li]
            # xg block (bf16, 256B rows) + all-gather
            xg_src = dramp.tile([B, XGW], BF16, tag="xgs", name=f"xg_src{li}")
            xg_full = dramp.tile([cfg.N, XGW], BF16, tag="xgf", addr_space="Shared",
                                 name=f"xg_full{li}")
            for nt, c0, pn in nodeblocks():
                px = pmm.tile([P, NHID], F32, tag="mm", name="px")
                nc.tensor.matmul(px[:pn, :], lhsT=hb_cur[:, c0:c0 + pn],
                                 rhs=Wg_s[:, li * NHID:(li + 1) * NHID],
                                 start=True, stop=True)
                xs = sbp.tile([P, XGW], BF16, tag="xs", name="xs")
                nc.scalar.copy(out=xs[:pn, :NHID], in_=px[:pn, :])
                nc.sync.dma_start(out=xg_src[c0:c0 + pn, :], in_=xs[:pn, :])
            nc.gpsimd.collective_compute(
                "AllGather", ALU.bypass,
                replica_groups=[list(range(C))],
                ins=[xg_src[:, :]],
                outs=[xg_full[:, :]],
            )

            # gaussian edge coefficients [P, T] (f32 chain, bf16 out)
            t1 = gp.tile([P, T], F32, tag="g1", name="g1")
            t2 = gp.tile([P, T], F32, tag="g2", name="g2")
            nc.vector.tensor_scalar(out=t1[:], in0=u_s[:], scalar1=sc["wp0"],
                                    scalar2=None, op0=ALU.mult)
            nc.vector.tensor_scalar(out=t2[:], in0=v_s[:], scalar1=sc["wp1"],
                                    scalar2=sc["bp"], op0=ALU.mult, op1=ALU.add)
            t3 = gp.tile([P, T], F32, tag="g1", name="g3")
            nc.vector.tensor_tensor(out=t3[:], in0=t1[:], in1=t2[:], op=ALU.add)
            t4 = gp.tile([P, T], F32, tag="g2", name="g4")
            nc.scalar.activation(out=t4[:], in_=t3[:], func=AF.Tanh)
            t4b = gp.tile([P, T], F32, tag="g1", name="g4b")
            nc.vector.tensor_scalar(out=t4b[:], in0=t4[:], scalar1=sc["neg_mu"],
                                    scalar2=None, op0=ALU.add)
            t5 = gp.tile([P, T], F32, tag="g2", name="g5")
            nc.scalar.activation(out=t5[:], in_=t4b[:], func=AF.Square)
            gauss_b = gaussp.tile([P, T], BF16, tag="gauss", name="gauss")
            nc.scalar.activation(out=gauss_b[:], in_=t5[:], func=AF.Exp,
                                 scale=sc["s2inv"])

            # edge aggregation per dest block
            h_new = hp.tile([NHID, B], F32, tag="h", name=f"h{li + 1}")
            hb_new = hbp.tile([NHID, B], BF16, tag="hb", name=f"hb{li + 1}")
            for nt, c0, pn in nodeblocks():
                Kb = K2[2 * nt] + K2[2 * nt + 1]
                off = toff[2 * nt]
                pa = pag.tile([P, NHID], F32, tag="pa", name="pa")
                nc.tensor.matmul(pa[:pn, :], lhsT=hb_cur[:, c0:c0 + pn],
                                 rhs=Wr_s[:, li * NHID:(li + 1) * NHID],
                                 start=True, stop=(Kb == 0))
                if Kb > 0:
                    # bulk one-hot build for all Kb tiles of this dest block
                    oh = ohp.tile([P, KbMax * P], BF16, tag="oh", name="oh")
                    sel = selp.tile([P, KbMax * P], BF16, tag="sel", name="sel")
                    r_b = R_s[:, :].unsqueeze(1).broadcast_to([P, Kb, P])
                    dl_b = dl_s[:, off:off + Kb].unsqueeze(2).broadcast_to([P, Kb, P])
                    g_b = gauss_b[:, off:off + Kb].unsqueeze(2).broadcast_to([P, Kb, P])
                    oh3 = oh[:, :Kb * P].rearrange("p (k e) -> p k e", e=P)
                    sel3 = sel[:, :Kb * P].rearrange("p (k e) -> p k e", e=P)
                    nc.vector.tensor_tensor(out=oh3, in0=r_b, in1=dl_b, op=ALU.is_equal)
                    nc.vector.tensor_tensor(out=sel3, in0=oh3, in1=g_b, op=ALU.mult)
                    kg = 0  # tile index within the block (gather order)
                    for h in (0, 1):
                        Kh = K2[2 * nt + h]
                        if Kh == 0:
                            continue
                        hoff = toff[2 * nt + h]
                        xj = xjp.tile([P, MAXT * XGW], BF16, tag="xj", name="xj")
                        xjs = []
                        for k0 in range(0, Kh, MAXT):
                            kc = min(MAXT, Kh - k0)
                            if k0 > 0:
                                xj = xjp.tile([P, MAXT * XGW], BF16, tag="xj",
                                              name="xj")
                            out_ap = xj[:, 0:kc * XGW].rearrange(
                                "p (k e) -> p k e", e=XGW)
                            nc.gpsimd.dma_gather(
                                out_ap, xg_full[h * HALF:(h + 1) * HALF, :],
                                idx_s[:, (hoff + k0) * 8:(hoff + k0 + kc) * 8],
                                kc * P, kc * P, XGW,
                                queue_num=(2 * nt + h) % 4)
                            xjs.append((xj, kc))
                        for ci, (xj, kc) in enumerate(xjs):
                            for k in range(kc):
                                t = kg + ci * MAXT + k
                                nc.tensor.matmul(
                                    pa[:pn, :],
                                    lhsT=sel[:, t * P:(t + 1) * P][:, :pn],
                                    rhs=xj[:, k * XGW:k * XGW + NHID],
                                    start=False, stop=(t == Kb - 1))
                        kg += Kh
                # epilogue: h_new = h_cur + relu(agg + Wroot h + bconv)
                et = sbp.tile([P, NHID], BF16, tag="et", name="et")
                nc.scalar.copy(out=et[:pn, :], in_=pa[:pn, :])
                pt2 = ptr.tile([NHID, P], BF16, tag="tr", name="pt2")
                nc.tensor.transpose(out=pt2[:, :pn], in_=et[:pn, :NHID],
                                    identity=id_s[:pn, :pn])
                rl = sbp.tile([NHID, P], F32, tag="rl", name="rl")
                nc.scalar.activation(out=rl[:, :pn], in_=pt2[:, :pn], func=AF.Relu,
                                     bias=bconv_a[:, li:li + 1])
                nc.vector.tensor_tensor(out=h_new[:, c0:c0 + pn], in0=rl[:, :pn],
                                        in1=h_cur[:, c0:c0 + pn], op=ALU.add)
                nc.vector.tensor_copy(out=hb_new[:, c0:c0 + pn],
                                      in_=h_new[:, c0:c0 + pn])
            h_cur, hb_cur = h_new, hb_new

        # ---- output head ----
        for nt, c0, pn in nodeblocks():
            po = pmm.tile([P, NHID], F32, tag="mm", name="po")
            nc.tensor.matmul(po[:pn, :NCLASS], lhsT=hb_cur[:, c0:c0 + pn], rhs=Wo_s[:],
                             start=True, stop=True)
            ob = sbp.tile([P, NCLASS], F32, tag="ob", name="ob")
            nc.vector.tensor_tensor(out=ob[:pn, :], in0=po[:pn, :NCLASS],
                                    in1=bout_v[:pn, :], op=ALU.add)
            nc.sync.dma_start(out=out_ext[c0:c0 + pn, :], in_=ob[:pn, :])

    nc.finalize()
    return nc


def make_in_maps(cfg, prep, h, W_emb, b_emb, Wg, Wroot, b_conv, W_out, b_out):
    C, B, NL, NHID = cfg.C, cfg.B, cfg.NL, cfg.NHID
    h = np.asarray(h, np.float32)
    R = np.tile(np.arange(P, dtype=np.float32), (P, 1))
    ident = np.eye(P, dtype=np.float32)
    bf = ml_dtypes.bfloat16
    common = dict(
        R=np.ascontiguousarray(R.astype(bf)),
        ident=np.ascontiguousarray(ident.astype(bf)),
        Wemb=np.ascontiguousarray(np.asarray(W_emb, np.float32).astype(bf)),
        Wg=np.ascontiguousarray(np.asarray(Wg, np.float32).reshape(NL, NHID, NHID).astype(bf)),
        Wr=np.ascontiguousarray(np.asarray(Wroot, np.float32).astype(bf)),
        Wo=np.ascontiguousarray(np.asarray(W_out, np.float32).astype(bf)),
        bemb=np.ascontiguousarray(np.asarray(b_emb, np.float32)[:, None]),
        bconv=np.ascontiguousarray(np.asarray(b_conv, np.float32).T),
        bout=np.ascontiguousarray(np.tile(np.asarray(b_out, np.float32), (P, 1))),
    )
    in_maps = []
    for m in range(C):
        d = dict(common)
        d["hT"] = np.ascontiguousarray(h[m * B:(m + 1) * B, :].T.astype(bf))
        d["idx16"] = np.ascontiguousarray(prep["idxA"][m])
        d["uv"] = np.ascontiguousarray(prep["uvA"][m])
        d["dl"] = np.ascontiguousarray(prep["dlA"][m].astype(bf))
        in_maps.append(d)
    return in_maps


def make_scal(cfg, Wp, bp, mu, sigma):
    Wp = np.asarray(Wp, np.float64)
    bp = np.asarray(bp, np.float64)
    mu = np.asarray(mu, np.float64)
    sigma = np.asarray(sigma, np.float64)
    out = []
    for i in range(cfg.NL):
        out.append(dict(
            wp0=float(Wp[i, 0, 0]),
            wp1=float(Wp[i, 1, 0]),
            bp=float(bp[i, 0]),
            neg_mu=float(-mu[i, 0, 0]),
            s2inv=float(-0.5 / (EPS + sigma[i, 0, 0] ** 2)),
        ))
    return out


def run(cfg, inputs, trace=False):
    prep = host_prep_dg(cfg, inputs["edge_index"], inputs["edge_weight"])
    scal = make_scal(cfg, inputs["Wp"], inputs["bp"], inputs["mu"], inputs["sigma"])
    nc = build(cfg, prep, scal)
    in_maps = make_in_maps(cfg, prep, inputs["h"], inputs["W_emb"], inputs["b_emb"],
                           inputs["Wg"], inputs["Wroot"], inputs["b_conv"],
                           inputs["W_out"], inputs["b_out"])
    res = bass_utils.run_bass_kernel_spmd(nc, in_maps, core_ids=list(range(cfg.C)),
                                          trace=trace)
    out = np.concatenate([res.results[m]["out"] for m in range(cfg.C)], axis=0)
    return out.astype(np.float32), res


def kernel(**inputs):
    cfg = Cfg()
    out, _ = run(cfg, inputs, trace=False)
    return out


# revision 16
# speedup vs baseline: 2.0190x; 1.1588x over previous
"""MoNet (GMMConv GNN) distributed Trainium2 kernel.

Strategy (8 NeuronCores):
  - Nodes partitioned into 8 contiguous blocks of B=6250 (core m owns dests
    [m*B,(m+1)*B)).  Edges bucketed by (dest block, source half) and padded to
    128-lane tiles, so each core's segment-sum over its dest block is local.
  - Per layer: each core computes its block of xg = h @ Wg in bf16 (rows
    padded to 128 cols = 256B), AllGather -> full bf16 xg table in DRAM, then
    per-edge gather of source rows via SWDGE dma_gather (int16 idx, negative
    trailing idx = skipped pad descriptors), gaussian-weighted segment-sum as
    one-hot bf16 matmuls accumulating in PSUM (dest blocks of 128 nodes),
    fused with the root-weight matmul; epilogue relu+bias+residual in
    transposed layout (h kept in f32, bf16 shadow for matmuls).
  - One-hot selection matrices are built in bulk per dest block with two
    broadcast tensor_tensor ops (is_equal with an iota-row table, then a
    per-tile gauss scale), instead of per-tile per-partition-pointer
    tensor_scalar ops which are slow on DVE.
  - Host does index prep only: degree/dinv, edge sorting/padding, per-core
    edge tables. All O(N*F) and O(E*F) math runs on device.
"""

import sys
from contextlib import ExitStack

import numpy as np

if "/opt/trn_rl_repo" not in sys.path:
    sys.path.insert(0, "/opt/trn_rl_repo")

import ml_dtypes

import concourse.bacc as bacc
import concourse.mybir as mybir
import concourse.tile as tile
from concourse import bass_utils

F32 = mybir.dt.float32
BF16 = mybir.dt.bfloat16
I16 = mybir.dt.int16
AF = mybir.ActivationFunctionType
ALU = mybir.AluOpType

P = 128
EPS = 1e-15


class Cfg:
    def __init__(self, N=50000, E=800000, NFEAT=128, NHID=96, NCLASS=40, NL=2, C=8):
        self.N, self.E, self.NFEAT, self.NHID, self.NCLASS = N, E, NFEAT, NHID, NCLASS
        self.NL, self.C = NL, C
        assert N % C == 0
        self.B = N // C
        self.NBLK = (self.B + P - 1) // P
        self.HALF = N // 2
        self.XGW = 128  # bf16 row padded to 256B


def host_prep_dg(cfg, edge_index, edge_weight):
    """Edges bucketed by (dest block, source half) for int16 dma_gather."""
    N, C, B, NBLK, HALF = cfg.N, cfg.C, cfg.B, cfg.NBLK, cfg.HALF
    row = np.asarray(edge_index[0]).astype(np.int64)
    col = np.asarray(edge_index[1]).astype(np.int64)
    ew = np.asarray(edge_weight).astype(np.float64)
    deg = np.bincount(row, weights=ew, minlength=N).astype(np.float32)
    with np.errstate(divide="ignore"):
        dinv = np.where(deg > 0, 1.0 / np.sqrt(deg.astype(np.float64)), 0.0).astype(np.float32)

    half = (row >= HALF).astype(np.int64)
    core = col // B
    loc = col - core * B
    blk = loc // P
    order = np.lexsort((half, blk, core))
    rs, cs = row[order], col[order]
    hs = half[order]
    core, loc, blk = core[order], loc[order], blk[order]
    dl = (loc - blk * P).astype(np.float32)

    NG = NBLK * 2
    g = blk * 2 + hs  # group within core
    cnt = np.zeros((C, NG), np.int64)
    np.add.at(cnt, (core, g), 1)
    K = ((cnt + P - 1) // P).max(axis=0)  # [NG] tiles per (blk, half)
    toff = np.concatenate([[0], np.cumsum(K)]).astype(np.int64)
    T = int(toff[-1])

    gg = core * NG + g
    gcnt = np.bincount(gg, minlength=C * NG)
    gstart = np.concatenate([[0], np.cumsum(gcnt)])[:-1]
    idx_in_g = np.arange(len(gg)) - gstart[gg]
    lane = (idx_in_g % P).astype(np.int64)
    tloc = idx_in_g // P  # tile within the (blk, half) group
    tcol = (toff[g] + tloc).astype(np.int64)

    uvA = np.zeros((C, P, 2 * T), np.float32)
    dlA = np.full((C, P, T), -1.0, np.float32)  # pad sentinel: never matches iota
    uvA[core, lane, tcol] = dinv[rs]
    uvA[core, lane, T + tcol] = dinv[cs]
    dlA[core, lane, tcol] = dl
    # int16 idx in wrapped-16 layout: flat k = tloc*128 + lane within a call;
    # element k at [k % 16, call_off*8 + k // 16]; pad = row 0 (valid; the
    # dl=-1 sentinel zeroes those lanes in the selection matrix).
    idxA = np.zeros((C, 16, 8 * T), np.int16)
    k = tloc * P + lane
    r16 = (k % 16).astype(np.int64)
    c16 = (toff[g] * 8 + k // 16).astype(np.int64)
    idxA[core, r16, c16] = (rs - hs * HALF).astype(np.int16)
    idxA = np.tile(idxA, (1, 8, 1))  # replicate 16-row block to 128 partitions
    return dict(idxA=idxA, uvA=uvA, dlA=dlA, K=[int(x) for x in K],
                toff=[int(x) for x in toff], T=T)


def build(cfg, prep, scal):
    """Build the SPMD Bass graph. scal: list of per-layer dicts with floats
    wp0, wp1, bp, neg_mu, s2inv."""
    NHID, NCLASS, NFEAT = cfg.NHID, cfg.NCLASS, cfg.NFEAT
    B, NBLK, NL, C, XGW = cfg.B, cfg.NBLK, cfg.NL, cfg.C, cfg.XGW
    T = prep["T"]
    HALF = cfg.HALF
    K2, toff = prep["K"], prep["toff"]
    Kmax = max(max(K2), 1)
    KbMax = max(K2[2 * i] + K2[2 * i + 1] for i in range(NBLK))
    MAXT = 7  # cap descriptors per call under the SWDGE ring size

    nc = bacc.Bacc("TRN2", target_bir_lowering=False, debug=False, num_devices=C,
                   num_swdge_queues=4, dynamic_dma_scratch_size=32768)
    hT_in = nc.declare_dram_parameter("hT", [NFEAT, B], BF16, isOutput=False)
    idx_in = nc.declare_dram_parameter("idx16", [P, 8 * T], I16, isOutput=False)
    uv_in = nc.declare_dram_parameter("uv", [P, 2 * T], F32, isOutput=False)
    dl_in = nc.declare_dram_parameter("dl", [P, T], BF16, isOutput=False)
    R_in = nc.declare_dram_parameter("R", [P, P], BF16, isOutput=False)
    id_in = nc.declare_dram_parameter("ident", [P, P], BF16, isOutput=False)
    Wemb_in = nc.declare_dram_parameter("Wemb", [NFEAT, NHID], BF16, isOutput=False)
    Wg_in = nc.declare_dram_parameter("Wg", [NL, NHID, NHID], BF16, isOutput=False)
    Wr_in = nc.declare_dram_parameter("Wr", [NL, NHID, NHID], BF16, isOutput=False)
    Wo_in = nc.declare_dram_parameter("Wo", [NHID, NCLASS], BF16, isOutput=False)
    bemb_in = nc.declare_dram_parameter("bemb", [NHID, 1], F32, isOutput=False)
    bconv_in = nc.declare_dram_parameter("bconv", [NHID, NL], F32, isOutput=False)
    bout_in = nc.declare_dram_parameter("bout", [P, NCLASS], F32, isOutput=False)
    out_ext = nc.declare_dram_parameter("out", [B, NCLASS], F32, isOutput=True)

    from concourse import library_config

    with tile.TileContext(nc) as tc, ExitStack() as ctx:
        nc.gpsimd.load_library(library_config.mlp)
        const = ctx.enter_context(tc.tile_pool(name="const", bufs=1))
        sbp = ctx.enter_context(tc.tile_pool(name="sbp", bufs=3))
        xjp = ctx.enter_context(tc.tile_pool(name="xjp", bufs=8))
        ohp = ctx.enter_context(tc.tile_pool(name="ohp", bufs=2))
        selp = ctx.enter_context(tc.tile_pool(name="selp", bufs=3))
        gp = ctx.enter_context(tc.tile_pool(name="gp", bufs=2))
        gaussp = ctx.enter_context(tc.tile_pool(name="gaussp", bufs=2))
        hp = ctx.enter_context(tc.tile_pool(name="hp", bufs=2))
        hbp = ctx.enter_context(tc.tile_pool(name="hbp", bufs=2))
        pag = ctx.enter_context(tc.tile_pool(name="pag", bufs=3, space="PSUM"))
        pmm = ctx.enter_context(tc.tile_pool(name="pmm", bufs=3, space="PSUM"))
        ptr = ctx.enter_context(tc.tile_pool(name="ptr", bufs=2, space="PSUM"))
        dramp = ctx.enter_context(tc.tile_pool(name="dramp", bufs=1, space="DRAM"))

        def cload(ap, shape, dtype=F32, name=None):
            t = const.tile(shape, dtype, name=name or "c")
            nc.sync.dma_start(out=t[:], in_=ap)
            return t

        hT_s = cload(hT_in[:, :], [NFEAT, B], BF16, name="hT_s")
        idx_s = cload(idx_in[:, :], [P, 8 * T], I16, name="idx_s")
        uv_s = cload(uv_in[:, :], [P, 2 * T], F32, name="uv_s")
        dl_s = cload(dl_in[:, :], [P, T], BF16, name="dl_s")
        u_s = uv_s[:, 0:T]
        v_s = uv_s[:, T:2 * T]
        R_s = cload(R_in[:, :], [P, P], BF16, name="R_s")
        id_s = cload(id_in[:, :], [P, P], BF16, name="id_s")
        Wemb_s = cload(Wemb_in[:, :], [NFEAT, NHID], BF16, name="Wemb_s")
        Wo_s = cload(Wo_in[:, :], [NHID, NCLASS], BF16, name="Wo_s")
        bemb_s = cload(bemb_in[:, :], [NHID, 1], F32, name="bemb_s")
        bconv_s = cload(bconv_in[:, :], [NHID, NL], F32, name="bconv_s")
        bout_s = cload(bout_in[:, :], [P, NCLASS], F32, name="bout_s")
        Wg_s = const.tile([NHID, NL * NHID], BF16, name="Wg_s")
        Wr_s = const.tile([NHID, NL * NHID], BF16, name="Wr_s")
        for i in range(NL):
            nc.sync.dma_start(out=Wg_s[:, i * NHID:(i + 1) * NHID], in_=Wg_in[i])
            nc.sync.dma_start(out=Wr_s[:, i * NHID:(i + 1) * NHID], in_=Wr_in[i])
        bconv_a = const.tile([NHID, NL], F32, name="bconv_a")
        nc.scalar.copy(out=bconv_a[:], in_=bconv_s[:])
        bemb_a = const.tile([NHID, 1], F32, name="bemb_a")
        nc.scalar.copy(out=bemb_a[:], in_=bemb_s[:])
        bout_v = const.tile([P, NCLASS], F32, name="bout_v")
        nc.vector.tensor_copy(out=bout_v[:], in_=bout_s[:])

        def nodeblocks():
            for nt in range(NBLK):
                c0 = nt * P
                yield nt, c0, min(P, B - c0)

        # ---- embedding: h0_T[96, B] = (h @ Wemb + bemb).T ----
        h_cur = hp.tile([NHID, B], F32, tag="h", name="h0")
        hb_cur = hbp.tile([NHID, B], BF16, tag="hb", name="hb0")
        for nt, c0, pn in nodeblocks():
            pe = pmm.tile([P, NHID], F32, tag="mm", name="pe")
            nc.tensor.matmul(pe[:pn, :], lhsT=hT_s[:, c0:c0 + pn], rhs=Wemb_s[:],
                             start=True, stop=True)
            tmp = sbp.tile([P, NHID], BF16, tag="embt", name="embt")
            nc.scalar.copy(out=tmp[:pn, :], in_=pe[:pn, :])
            pt = ptr.tile([NHID, P], BF16, tag="tr", name="pt")
            nc.tensor.transpose(out=pt[:, :pn], in_=tmp[:pn, :NHID], identity=id_s[:pn, :pn])
            nc.scalar.activation(out=h_cur[:, c0:c0 + pn], in_=pt[:, :pn],
                                 func=AF.Identity, bias=bemb_a[:, :1])
            nc.scalar.copy(out=hb_cur[:, c0:c0 + pn], in_=h_cur[:, c0:c0 + pn])

        # ---- layers ----
        for li in range(NL):
            sc = scal[li]
            # xg block (bf16, 256B rows) + all-gather
            xg_src = dramp.tile([B, XGW], BF16, tag="xgs", name=f"xg_src{li}")
            xg_full = dramp.tile([cfg.N, XGW], BF16, tag="xgf", addr_space="Shared",
                                 name=f"xg_full{li}")
            for nt, c0, pn in nodeblocks():
                px = pmm.tile([P, NHID], F32, tag="mm", name="px")
                nc.tensor.matmul(px[:pn, :], lhsT=hb_cur[:, c0:c0 + pn],
                                 rhs=Wg_s[:, li * NHID:(li + 1) * NHID],
                                 start=True, stop=True)
                xs = sbp.tile([P, XGW], BF16, tag="xs", name="xs")
                nc.scalar.copy(out=xs[:pn, :NHID], in_=px[:pn, :])
                nc.sync.dma_start(out=xg_src[c0:c0 + pn, :], in_=xs[:pn, :])
            nc.gpsimd.collective_compute(
                "AllGather", ALU.bypass,
                replica_groups=[list(range(C))],
                ins=[xg_src[:, :]],
                outs=[xg_full[:, :]],
            )

            # gaussian edge coefficients [P, T] (f32 chain, bf16 out)
            t1 = gp.tile([P, T], F32, tag="g1", name="g1")
            t2 = gp.tile([P, T], F32, tag="g2", name="g2")
            nc.vector.tensor_scalar(out=t1[:], in0=u_s[:], scalar1=sc["wp0"],
                                    scalar2=None, op0=ALU.mult)
            nc.vector.tensor_scalar(out=t2[:], in0=v_s[:], scalar1=sc["wp1"],
                                    scalar2=sc["bp"], op0=ALU.mult, op1=ALU.add)
            t3 = gp.tile([P, T], F32, tag="g1", name="g3")
            nc.vector.tensor_tensor(out=t3[:], in0=t1[:], in1=t2[:], op=ALU.add)
            t4 = gp.tile([P, T], F32, tag="g2", name="g4")
            nc.scalar.activation(out=t4[:], in_=t3[:], func=AF.Tanh)
            t4b = gp.tile([P, T], F32, tag="g1", name="g4b")
            nc.vector.tensor_scalar(out=t4b[:], in0=t4[:], scalar1=sc["neg_mu"],
                                    scalar2=None, op0=ALU.add)
            t5 = gp.tile([P, T], F32, tag="g2", name="g5")
            nc.scalar.activation(out=t5[:], in_=t4b[:], func=AF.Square)
            gauss_b = gaussp.tile([P, T], BF16, tag="gauss", name="gauss")
            nc.scalar.activation(out=gauss_b[:], in_=t5[:], func=AF.Exp,
                                 scale=sc["s2inv"])

            # edge aggregation per dest block
            h_new = hp.tile([NHID, B], F32, tag="h", name=f"h{li + 1}")
            hb_new = hbp.tile([NHID, B], BF16, tag="hb", name=f"hb{li + 1}")
            gq = 0  # running gather-call counter for queue rotation
            for nt, c0, pn in nodeblocks():
                Kb = K2[2 * nt] + K2[2 * nt + 1]
                off = toff[2 * nt]
                pa = pag.tile([P, NHID], F32, tag="pa", name="pa")
                nc.tensor.matmul(pa[:pn, :], lhsT=hb_cur[:, c0:c0 + pn],
                                 rhs=Wr_s[:, li * NHID:(li + 1) * NHID],
                                 start=True, stop=(Kb == 0))
                if Kb > 0:
                    # bulk one-hot build for all Kb tiles of this dest block
                    oh = ohp.tile([P, KbMax * P], BF16, tag="oh", name="oh")
                    sel = selp.tile([P, KbMax * P], BF16, tag="sel", name="sel")
                    r_b = R_s[:, :].unsqueeze(1).broadcast_to([P, Kb, P])
                    dl_b = dl_s[:, off:off + Kb].unsqueeze(2).broadcast_to([P, Kb, P])
                    g_b = gauss_b[:, off:off + Kb].unsqueeze(2).broadcast_to([P, Kb, P])
                    oh3 = oh[:, :Kb * P].rearrange("p (k e) -> p k e", e=P)
                    sel3 = sel[:, :Kb * P].rearrange("p (k e) -> p k e", e=P)
                    nc.vector.tensor_tensor(out=oh3, in0=r_b, in1=dl_b, op=ALU.is_equal)
                    nc.vector.tensor_tensor(out=sel3, in0=oh3, in1=g_b, op=ALU.mult)
                    kg = 0  # tile index within the block (gather order)
                    for h in (0, 1):
                        Kh = K2[2 * nt + h]
                        if Kh == 0:
                            continue
                        hoff = toff[2 * nt + h]
                        xj = xjp.tile([P, MAXT * XGW], BF16, tag="xj", name="xj")
                        xjs = []
                        for k0 in range(0, Kh, MAXT):
                            kc = min(MAXT, Kh - k0)
                            if k0 > 0:
                                xj = xjp.tile([P, MAXT * XGW], BF16, tag="xj",
                                              name="xj")
                            out_ap = xj[:, 0:kc * XGW].rearrange(
                                "p (k e) -> p k e", e=XGW)
                            nc.gpsimd.dma_gather(
                                out_ap, xg_full[h * HALF:(h + 1) * HALF, :],
                                idx_s[:, (hoff + k0) * 8:(hoff + k0 + kc) * 8],
                                kc * P, kc * P, XGW,
                                queue_num=gq % 4)
                            gq += 1
                            xjs.append((xj, kc))
                        for ci, (xj, kc) in enumerate(xjs):
                            for k in range(kc):
                                t = kg + ci * MAXT + k
                                nc.tensor.matmul(
                                    pa[:pn, :],
                                    lhsT=sel[:, t * P:(t + 1) * P][:, :pn],
                                    rhs=xj[:, k * XGW:k * XGW + NHID],
                                    start=False, stop=(t == Kb - 1))
                        kg += Kh
                # epilogue: h_new = h_cur + relu(agg + Wroot h + bconv)
                et = sbp.tile([P, NHID], BF16, tag="et", name="et")
                nc.scalar.copy(out=et[:pn, :], in_=pa[:pn, :])
                pt2 = ptr.tile([NHID, P], BF16, tag="tr", name="pt2")
                nc.tensor.transpose(out=pt2[:, :pn], in_=et[:pn, :NHID],
                                    identity=id_s[:pn, :pn])
                rl = sbp.tile([NHID, P], F32, tag="rl", name="rl")
                nc.scalar.activation(out=rl[:, :pn], in_=pt2[:, :pn], func=AF.Relu,
                                     bias=bconv_a[:, li:li + 1])
                nc.vector.tensor_tensor(out=h_new[:, c0:c0 + pn], in0=rl[:, :pn],
                                        in1=h_cur[:, c0:c0 + pn], op=ALU.add)
                nc.scalar.copy(out=hb_new[:, c0:c0 + pn],
                               in_=h_new[:, c0:c0 + pn])
            h_cur, hb_cur = h_new, hb_new

        # ---- output head ----
        for nt, c0, pn in nodeblocks():
            po = pmm.tile([P, NHID], F32, tag="mm", name="po")
            nc.tensor.matmul(po[:pn, :NCLASS], lhsT=hb_cur[:, c0:c0 + pn], rhs=Wo_s[:],
                             start=True, stop=True)
            ob = sbp.tile([P, NCLASS], F32, tag="ob", name="ob")
            nc.vector.tensor_tensor(out=ob[:pn, :], in0=po[:pn, :NCLASS],
                                    in1=bout_v[:pn, :], op=ALU.add)
            nc.sync.dma_start(out=out_ext[c0:c0 + pn, :], in_=ob[:pn, :])

    nc.finalize()
    return nc


def make_in_maps(cfg, prep, h, W_emb, b_emb, Wg, Wroot, b_conv, W_out, b_out):
    C, B, NL, NHID = cfg.C, cfg.B, cfg.NL, cfg.NHID
    h = np.asarray(h, np.float32)
    R = np.tile(np.arange(P, dtype=np.float32), (P, 1))
    ident = np.eye(P, dtype=np.float32)
    bf = ml_dtypes.bfloat16
    common = dict(
        R=np.ascontiguousarray(R.astype(bf)),
        ident=np.ascontiguousarray(ident.astype(bf)),
        Wemb=np.ascontiguousarray(np.asarray(W_emb, np.float32).astype(bf)),
        Wg=np.ascontiguousarray(np.asarray(Wg, np.float32).reshape(NL, NHID, NHID).astype(bf)),
        Wr=np.ascontiguousarray(np.asarray(Wroot, np.float32).astype(bf)),
        Wo=np.ascontiguousarray(np.asarray(W_out, np.float32).astype(bf)),
        bemb=np.ascontiguousarray(np.asarray(b_emb, np.float32)[:, None]),
        bconv=np.ascontiguousarray(np.asarray(b_conv, np.float32).T),
        bout=np.ascontiguousarray(np.tile(np.asarray(b_out, np.float32), (P, 1))),
    )
    in_maps = []
    for m in range(C):
        d = dict(common)
        d["hT"] = np.ascontiguousarray(h[m * B:(m + 1) * B, :].T.astype(bf))
        d["idx16"] = np.ascontiguousarray(prep["idxA"][m])
        d["uv"] = np.ascontiguousarray(prep["uvA"][m])
        d["dl"] = np.ascontiguousarray(prep["dlA"][m].astype(bf))
        in_maps.append(d)
    return in_maps


def make_scal(cfg, Wp, bp, mu, sigma):
    Wp = np.asarray(Wp, np.float64)
    bp = np.asarray(bp, np.float64)
    mu = np.asarray(mu, np.float64)
    sigma = np.asarray(sigma, np.float64)
    out = []
    for i in range(cfg.NL):
        out.append(dict(
            wp0=float(Wp[i, 0, 0]),
            wp1=float(Wp[i, 1, 0]),
            bp=float(bp[i, 0]),
            neg_mu=float(-mu[i, 0, 0]),
            s2inv=float(-0.5 / (EPS + sigma[i, 0, 0] ** 2)),
        ))
    return out


def run(cfg, inputs, trace=False):
    prep = host_prep_dg(cfg, inputs["edge_index"], inputs["edge_weight"])
    scal = make_scal(cfg, inputs["Wp"], inputs["bp"], inputs["mu"], inputs["sigma"])
    nc = build(cfg, prep, scal)
    in_maps = make_in_maps(cfg, prep, inputs["h"], inputs["W_emb"], inputs["b_emb"],
                           inputs["Wg"], inputs["Wroot"], inputs["b_conv"],
                           inputs["W_out"], inputs["b_out"])
    res = bass_utils.run_bass_kernel_spmd(nc, in_maps, core_ids=list(range(cfg.C)),
                                          trace=trace)
    out = np.concatenate([res.results[m]["out"] for m in range(cfg.C)], axis=0)
    return out.astype(np.float32), res


def kernel(**inputs):
    cfg = Cfg()
    out, _ = run(cfg, inputs, trace=False)
    return out
